# revision 21
# baseline (speedup 1.0000x reference)
"""DIST loss (hard CE + inter/intra Pearson distillation) on 8 Trainium2 cores.

Strategy: data-parallel over the batch dim (4096 rows -> 512 rows/core), with
z_s/z_t cast to bf16 on the host (halves HBM traffic; exp/product sums tolerate
the 2^-8 mantissa easily at the 2e-2 gate).

Per 128-row block each core streams its [128, 32000] bf16 shards once:
  - ScalarE: exp on 8000-wide tiles (accum_out -> per-row Zs/Zt) plus 5 of the
    8 per-block es^2 Square tiles (accum_out -> U11 partials).
  - VectorE: remaining es^2 tiles, et^2, es*et products (bf16 2x mode), the
    2000->1000 halve-add tops of the U22/U12/U11 row-sum chains, reciprocals
    and the 5 weight columns.
  - GpSimd: 1000->500 halve + final 500-col reduces of the row-sum chains, and
    PSUM->SBUF bf16 evacuation of the column-stat matmul results.
  - TensorE: per 2000-col chunk, 4 sub-matmuls (512/512/512/464 wide) x 5
    accumulating stats with per-stat weight columns (1/Zs, 1/Zt, 1/Zs^2,
    1/Zt^2, 1/(Zs*Zt)) as the stationary operand, landing at PSUM partition
    bases 0/32/64/96 of one [128, 512] bank.
The host sums the bf16 column-stat partials over blocks/cores and finishes the
O(B + C) scalar math (Pearson means, label gather, log) in float64; the hard-CE
label gather uses the original f32 z_s.
"""
import sys
import types
import numpy as np

sys.path.insert(0, "/opt/trn_rl_repo")

B, C = 4096, 32000
N_CORES = 8
R = B // N_CORES          # 512 rows per core
P = 128                   # partitions
NBLK = R // P             # 4 row blocks per core
TW = 8000                 # exp tile width
NT = C // TW              # 4 exp tiles per block per tensor
PW = 4000                 # product tile width
NP = C // PW              # 8 product tiles per block
MW = 2000                 # psum chunk width
NM = C // MW              # 16 psum chunks per block
SUBS = [(0, 512), (512, 512), (1024, 512), (1536, 464)]
K_SQ = 6                  # product tiles whose es^2 runs on ScalarE (accum U11)
A22 = 5                   # product tiles whose et^2 chain top runs on GpSimd
EPS = 1e-8

_built = None


def _install_ntff_shim():
    # antenv.axon_hooks is absent in this image; register the ctypes NTFF
    # hook so run_bass_kernel_spmd(trace=True) can profile under axon.
    try:
        import antenv
        import trn_agent_boot.trn_boot as tb
        if "antenv.axon_hooks" in sys.modules:
            return
        hook = tb._ntff_profile_via_ctypes("/opt/axon/libaxon_pjrt.so")
        mod = types.ModuleType("antenv.axon_hooks")
        mod.get_axon_ntff_profile_hook = lambda: hook
        mod.set_axon_ntff_profile_hook = lambda h: None
        antenv.axon_hooks = mod
        sys.modules["antenv.axon_hooks"] = mod
    except Exception:
        pass


def _build():
    from contextlib import ExitStack
    import concourse.bacc as bacc
    import concourse.tile as tile
    from concourse import mybir

    f32 = mybir.dt.float32
    bf16 = mybir.dt.bfloat16
    Exp = mybir.ActivationFunctionType.Exp
    Square = mybir.ActivationFunctionType.Square
    ADD = mybir.AluOpType.add
    MULT = mybir.AluOpType.mult
    AXF = mybir.AxisListType.X

    nc = bacc.Bacc("TRN2", target_bir_lowering=False, debug=False)
    zs_d = nc.dram_tensor("z_s", [R, C], bf16, kind="ExternalInput")
    zt_d = nc.dram_tensor("z_t", [R, C], bf16, kind="ExternalInput")
    # per (block, 4000-col chunk pair): the full [128, 512] psum bank in bf16;
    # host picks rows 32*s + 8*h + k (sub s, half h, stat k), rest is garbage.
    col_d = nc.dram_tensor("colstats", [NBLK, NP, P, 512], bf16,
                           kind="ExternalOutput")
    row_d = nc.dram_tensor("rowstats", [R, 8], f32, kind="ExternalOutput")

    with tile.TileContext(nc) as tc, ExitStack() as ctx:
        zin = ctx.enter_context(tc.tile_pool(name="zin", bufs=2))
        esp = ctx.enter_context(tc.tile_pool(name="esp", bufs=NT))
        etp = ctx.enter_context(tc.tile_pool(name="etp", bufs=NT))
        prod = ctx.enter_context(tc.tile_pool(name="prod", bufs=4))
        h1p = ctx.enter_context(tc.tile_pool(name="h1p", bufs=2))
        h2p = ctx.enter_context(tc.tile_pool(name="h2p", bufs=2))
        h3p = ctx.enter_context(tc.tile_pool(name="h3p", bufs=2))
        h4p = ctx.enter_context(tc.tile_pool(name="h4p", bufs=2))
        stp = ctx.enter_context(tc.tile_pool(name="stp", bufs=3))
        wtp = ctx.enter_context(tc.tile_pool(name="wtp", bufs=2))
        small = ctx.enter_context(tc.tile_pool(name="small", bufs=2))
        psump = ctx.enter_context(tc.tile_pool(name="psum", bufs=6, space="PSUM"))

        # sync-queue DMA dispatch order is program order; stagger the output
        # DMAs one block behind the input DMAs so next-block input dispatch
        # never waits on this block's evacuations.
        out_dma_q = []

        for b in range(NBLK):
            r0 = b * P
            zsp = small.tile([P, NT], f32, tag="zsp")
            ztp = small.tile([P, NT], f32, tag="ztp")
            u11p = small.tile([P, NP], f32, tag="u11p")
            u22p = small.tile([P, NP], f32, tag="u22p")
            u12p = small.tile([P, NP], f32, tag="u12p")

            es_tiles = []
            et_tiles = []
            for t in range(NT):
                c0 = t * TW
                zs = zin.tile([P, TW], bf16, tag="zin")
                nc.sync.dma_start(zs[:], zs_d[r0:r0 + P, c0:c0 + TW])
                es = esp.tile([P, TW], bf16, tag="es")
                nc.scalar.activation(es[:], zs[:], Exp, accum_out=zsp[:, t:t + 1])
                zt = zin.tile([P, TW], bf16, tag="zin")
                nc.sync.dma_start(zt[:], zt_d[r0:r0 + P, c0:c0 + TW])
                et = etp.tile([P, TW], bf16, tag="et")
                nc.scalar.activation(et[:], zt[:], Exp, accum_out=ztp[:, t:t + 1])
                es_tiles.append(es)
                et_tiles.append(et)

            # flush the previous block's output DMAs now (after this block's
            # input dispatch) to keep the sync queue from stalling inputs.
            for fn in out_dma_q:
                fn()
            out_dma_q = []

            rs = small.tile([P, 8], f32, tag="rs")
            nc.vector.tensor_reduce(rs[:, 0:1], zsp[:, 0:NT], axis=AXF, op=ADD)
            nc.vector.tensor_reduce(rs[:, 1:2], ztp[:, 0:NT], axis=AXF, op=ADD)
            w1 = small.tile([P, 1], f32, tag="w1")
            nc.vector.reciprocal(w1[:], rs[:, 0:1])
            w2 = small.tile([P, 1], f32, tag="w2")
            nc.vector.reciprocal(w2[:], rs[:, 1:2])
            # W tiles are 13 columns wide: stat k of chunk-half h lives in
            # column 8*h + k, so a pair of 2000-col chunks shares one
            # [128, 512] psum tile (rows 32s+8h+k), halving evacuations.
            W_tiles = []
            for h in range(2):
                row = []
                for k in range(5):
                    Wk = wtp.tile([P, 16], bf16, tag=f"W{h}{k}")
                    nc.gpsimd.memset(Wk[:], 0.0)
                    row.append(Wk)
                c = 8 * h
                nc.gpsimd.tensor_copy(row[0][:, c:c + 1], w1[:])
                nc.gpsimd.tensor_copy(row[1][:, c + 1:c + 2], w2[:])
                nc.gpsimd.tensor_mul(row[2][:, c + 2:c + 3], w1[:], w1[:])
                nc.gpsimd.tensor_mul(row[3][:, c + 3:c + 4], w2[:], w2[:])
                nc.gpsimd.tensor_mul(row[4][:, c + 4:c + 5], w1[:], w2[:])
                W_tiles.append(row)

            # evacuations lag the matmuls by a couple of chunks so the copy
            # never stalls the issuing engine's in-order stream.
            pending_evac = []

            def flush_evac(n):
                while len(pending_evac) > n:
                    pending_evac.pop(0)()

            def chain(p, col, upcol, all_g):
                # row-sum of a [P, PW] bf16 product tile into upcol[:, col]
                e1 = nc.gpsimd if all_g else nc.vector
                h1 = h1p.tile([P, PW // 2], bf16, tag="h1")
                e1.tensor_add(h1[:], p[:, 0:PW // 2], p[:, PW // 2:PW])
                h2 = h2p.tile([P, PW // 4], bf16, tag="h2")
                e1.tensor_add(h2[:], h1[:, 0:PW // 4], h1[:, PW // 4:PW // 2])
                h3 = h3p.tile([P, PW // 8], bf16, tag="h3")
                nc.gpsimd.tensor_add(h3[:], h2[:, 0:PW // 8], h2[:, PW // 8:PW // 4])
                h4 = h4p.tile([P, PW // 16], bf16, tag="h4")
                nc.gpsimd.tensor_add(h4[:], h3[:, 0:PW // 16], h3[:, PW // 16:PW // 8])
                nc.vector.tensor_reduce(upcol[:, col:col + 1], h4[:], axis=AXF,
                                        op=ADD)

            for j in range(NP):
                t, half = j // 2, j % 2
                es_sl = es_tiles[t][:, half * PW:(half + 1) * PW]
                et_sl = et_tiles[t][:, half * PW:(half + 1) * PW]
                p11 = prod.tile([P, PW], bf16, tag="prod")
                if j < K_SQ:
                    nc.scalar.activation(p11[:], es_sl, Square,
                                         accum_out=u11p[:, j:j + 1])
                else:
                    nc.vector.tensor_mul(p11[:], es_sl, es_sl)
                p22 = prod.tile([P, PW], bf16, tag="prod")
                nc.vector.tensor_mul(p22[:], et_sl, et_sl)
                p12 = prod.tile([P, PW], bf16, tag="prod")
                nc.vector.scalar_tensor_tensor(
                    p12[:], es_sl, 1.0, et_sl, op0=MULT, op1=MULT,
                    accum_out=u12p[:, j:j + 1])
                if j >= K_SQ:
                    chain(p11, j, u11p, all_g=False)
                chain(p22, j, u22p, all_g=(j < A22))

                ps = psump.tile([P, 512], f32, tag="ps")
                for hh in range(2):
                    off = hh * MW
                    rhs = [es_sl, et_sl, p11, p22, p12]
                    for s, (o, w) in enumerate(SUBS):
                        for k in range(5):
                            nc.tensor.matmul(ps[32 * s:32 * s + 13, 0:w],
                                             W_tiles[hh][k][:, 0:13],
                                             rhs[k][:, off + o:off + o + w],
                                             start=(hh == 0 and k == 0),
                                             stop=(hh == 1 and k == 4),
                                             tile_position=(0, 32 * s))

                def emit_evac(bb=b, jj=j, pss=ps):
                    st = stp.tile([P, 512], bf16, tag="st")
                    nc.vector.tensor_copy(st[:], pss[:])

                    def emit_out():
                        nc.sync.dma_start(col_d[bb, jj], st[:])
                    out_dma_q.append(emit_out)
                pending_evac.append(emit_evac)
                flush_evac(2)
            flush_evac(0)

            nc.vector.tensor_reduce(rs[:, 2:3], u11p[:, 0:NP], axis=AXF, op=ADD)
            nc.vector.tensor_reduce(rs[:, 3:4], u22p[:, 0:NP], axis=AXF, op=ADD)
            nc.vector.tensor_reduce(rs[:, 4:5], u12p[:, 0:NP], axis=AXF, op=ADD)

            def emit_rs(bb=b, rss=rs):
                nc.sync.dma_start(row_d[bb * P:bb * P + P, 0:8], rss[:])
            out_dma_q.append(emit_rs)

        for fn in out_dma_q:
            fn()

    nc.compile()
    return nc


def _get_built():
    global _built
    if _built is None:
        _install_ntff_shim()
        _built = _build()
    return _built


def _unpack_col(colstats):
    """colstats [NBLK, NP, 128, 512] bf16 -> [5, C] float64 column stats."""
    acc = np.asarray(colstats).astype(np.float64).sum(axis=0)  # [NP, 128, 512]
    col = np.zeros((5, C), np.float64)
    for j in range(NP):
        for h in range(2):
            c0 = j * PW + h * MW
            for s, (o, w) in enumerate(SUBS):
                for k in range(5):
                    col[k, c0 + o:c0 + o + w] += acc[j, 32 * s + 8 * h + k, 0:w]
    return col


def run_sharded(z_s, z_t, trace=False, tmpdir=None):
    """Run the device program; returns (colstats_sum [5, C] f64,
    rowstats [B, 5] f64, BassKernelResults)."""
    import ml_dtypes
    from concourse.bass_utils import run_bass_kernel_spmd

    nc = _get_built()
    bf16 = ml_dtypes.bfloat16
    z_s = np.ascontiguousarray(np.asarray(z_s, dtype=np.float32).astype(bf16))
    z_t = np.ascontiguousarray(np.asarray(z_t, dtype=np.float32).astype(bf16))
    in_maps = [
        {"z_s": z_s[i * R:(i + 1) * R], "z_t": z_t[i * R:(i + 1) * R]}
        for i in range(N_CORES)
    ]
    res = run_bass_kernel_spmd(nc, in_maps, core_ids=list(range(N_CORES)),
                               trace=trace, tmpdir=tmpdir)
    col = np.zeros((5, C), np.float64)
    rows = []
    for i in range(N_CORES):
        col += _unpack_col(res.results[i]["colstats"])
        rows.append(res.results[i]["rowstats"][:, :5].astype(np.float64))
    return col, np.concatenate(rows, axis=0), res


def kernel(z_s, z_t, labels):
    col, rowstats, _ = run_sharded(z_s, z_t)
    return _finish(np.asarray(z_s), np.asarray(labels), col, rowstats)


def _finish(z_s, labels, col, rowstats):
    Zs, Zt, U11, U22, U12 = rowstats.T
    invC = 1.0 / C
    # inter: Pearson over classes per row (softmax rows have mean 1/C)
    num = U12 / (Zs * Zt) - invC
    vs = U11 / (Zs * Zs) - invC
    vt = U22 / (Zt * Zt) - invC
    corr = num / (np.sqrt(vs) * np.sqrt(vt) + EPS)
    inter = 1.0 - corr.mean()
    # intra: Pearson over samples per column
    S1, S2, S11, S22, S12 = col
    numc = S12 - S1 * S2 / B
    vsc = S11 - S1 * S1 / B
    vtc = S22 - S2 * S2 / B
    corrc = numc / (np.sqrt(vsc) * np.sqrt(vtc) + EPS)
    intra = 1.0 - corrc.mean()
    # hard CE: mean(logsumexp(z_s) - z_s[label])
    lab = np.asarray(labels).astype(np.int64).ravel()
    zl = z_s[np.arange(B), lab].astype(np.float64)
    hard = (np.log(Zs) - zl).mean()
    return np.float32(hard + inter + intra)


# revision 25
# speedup vs baseline: 1.0340x; 1.0340x over previous
"""DIST loss (hard CE + inter/intra Pearson distillation) on 8 Trainium2 cores.

Strategy: data-parallel over the batch dim (4096 rows -> 512 rows/core), with
z_s/z_t cast to bf16 on the host (halves HBM traffic; exp/product sums tolerate
the 2^-8 mantissa easily at the 2e-2 gate).

Per 128-row block each core streams its [128, 32000] bf16 shards once:
  - ScalarE: exp on 8000-wide tiles (accum_out -> per-row Zs/Zt) plus 5 of the
    8 per-block es^2 Square tiles (accum_out -> U11 partials).
  - VectorE: remaining es^2 tiles, et^2, es*et products (bf16 2x mode), the
    2000->1000 halve-add tops of the U22/U12/U11 row-sum chains, reciprocals
    and the 5 weight columns.
  - GpSimd: 1000->500 halve + final 500-col reduces of the row-sum chains, and
    PSUM->SBUF bf16 evacuation of the column-stat matmul results.
  - TensorE: per 2000-col chunk, 4 sub-matmuls (512/512/512/464 wide) x 5
    accumulating stats with per-stat weight columns (1/Zs, 1/Zt, 1/Zs^2,
    1/Zt^2, 1/(Zs*Zt)) as the stationary operand, landing at PSUM partition
    bases 0/32/64/96 of one [128, 512] bank.
The host sums the bf16 column-stat partials over blocks/cores and finishes the
O(B + C) scalar math (Pearson means, label gather, log) in float64; the hard-CE
label gather uses the original f32 z_s.
"""
import sys
import types
import numpy as np

sys.path.insert(0, "/opt/trn_rl_repo")

B, C = 4096, 32000
N_CORES = 8
R = B // N_CORES          # 512 rows per core
P = 128                   # partitions
NBLK = R // P             # 4 row blocks per core
TW = 8000                 # exp tile width
NT = C // TW              # 4 exp tiles per block per tensor
PW = 4000                 # product tile width
NP = C // PW              # 8 product tiles per block
MW = 2000                 # psum chunk width
NM = C // MW              # 16 psum chunks per block
SUBS = [(0, 512), (512, 512), (1024, 512), (1536, 464)]
K_SQ = 6                  # product tiles whose es^2 runs on ScalarE (accum U11)
A22 = 4                   # product tiles whose et^2 chain top runs on GpSimd
EPS = 1e-8

_built = None


def _install_ntff_shim():
    # antenv.axon_hooks is absent in this image; register the ctypes NTFF
    # hook so run_bass_kernel_spmd(trace=True) can profile under axon.
    try:
        import antenv
        import trn_agent_boot.trn_boot as tb
        if "antenv.axon_hooks" in sys.modules:
            return
        hook = tb._ntff_profile_via_ctypes("/opt/axon/libaxon_pjrt.so")
        mod = types.ModuleType("antenv.axon_hooks")
        mod.get_axon_ntff_profile_hook = lambda: hook
        mod.set_axon_ntff_profile_hook = lambda h: None
        antenv.axon_hooks = mod
        sys.modules["antenv.axon_hooks"] = mod
    except Exception:
        pass


def _build():
    from contextlib import ExitStack
    import concourse.bacc as bacc
    import concourse.tile as tile
    from concourse import mybir

    f32 = mybir.dt.float32
    bf16 = mybir.dt.bfloat16
    Exp = mybir.ActivationFunctionType.Exp
    Square = mybir.ActivationFunctionType.Square
    ADD = mybir.AluOpType.add
    MULT = mybir.AluOpType.mult
    AXF = mybir.AxisListType.X

    nc = bacc.Bacc("TRN2", target_bir_lowering=False, debug=False)
    zs_d = nc.dram_tensor("z_s", [R, C], bf16, kind="ExternalInput")
    zt_d = nc.dram_tensor("z_t", [R, C], bf16, kind="ExternalInput")
    # per (block, 4000-col chunk pair): the full [128, 512] psum bank in bf16;
    # host picks rows 32*s + 8*h + k (sub s, half h, stat k), rest is garbage.
    col_d = nc.dram_tensor("colstats", [NBLK, NP, P, 512], bf16,
                           kind="ExternalOutput")
    row_d = nc.dram_tensor("rowstats", [R, 8], f32, kind="ExternalOutput")

    with tile.TileContext(nc) as tc, ExitStack() as ctx:
        zin = ctx.enter_context(tc.tile_pool(name="zin", bufs=2))
        esp = ctx.enter_context(tc.tile_pool(name="esp", bufs=NT))
        etp = ctx.enter_context(tc.tile_pool(name="etp", bufs=NT))
        prod = ctx.enter_context(tc.tile_pool(name="prod", bufs=4))
        h1p = ctx.enter_context(tc.tile_pool(name="h1p", bufs=2))
        h2p = ctx.enter_context(tc.tile_pool(name="h2p", bufs=2))
        h3p = ctx.enter_context(tc.tile_pool(name="h3p", bufs=2))
        h4p = ctx.enter_context(tc.tile_pool(name="h4p", bufs=2))
        stp = ctx.enter_context(tc.tile_pool(name="stp", bufs=3))
        wtp = ctx.enter_context(tc.tile_pool(name="wtp", bufs=2))
        small = ctx.enter_context(tc.tile_pool(name="small", bufs=2))
        psump = ctx.enter_context(tc.tile_pool(name="psum", bufs=6, space="PSUM"))

        # sync-queue DMA dispatch order is program order; stagger the output
        # DMAs one block behind the input DMAs so next-block input dispatch
        # never waits on this block's evacuations.
        out_dma_q = []

        for b in range(NBLK):
            r0 = b * P
            zsp = small.tile([P, NT], f32, tag="zsp")
            ztp = small.tile([P, NT], f32, tag="ztp")
            u11p = small.tile([P, NP], f32, tag="u11p")
            u22p = small.tile([P, NP], f32, tag="u22p")
            u12p = small.tile([P, NP], f32, tag="u12p")

            es_tiles = []
            et_tiles = []
            for t in range(NT):
                c0 = t * TW
                zs = zin.tile([P, TW], bf16, tag="zin")
                nc.sync.dma_start(zs[:], zs_d[r0:r0 + P, c0:c0 + TW])
                es = esp.tile([P, TW], bf16, tag="es")
                nc.scalar.activation(es[:], zs[:], Exp, accum_out=zsp[:, t:t + 1])
                zt = zin.tile([P, TW], bf16, tag="zin")
                nc.sync.dma_start(zt[:], zt_d[r0:r0 + P, c0:c0 + TW])
                et = etp.tile([P, TW], bf16, tag="et")
                nc.scalar.activation(et[:], zt[:], Exp, accum_out=ztp[:, t:t + 1])
                es_tiles.append(es)
                et_tiles.append(et)

            # flush the previous block's output DMAs now (after this block's
            # input dispatch) to keep the sync queue from stalling inputs.
            for fn in out_dma_q:
                fn()
            out_dma_q = []

            # evacuations lag the matmuls by a couple of chunks so the copy
            # never stalls the issuing engine's in-order stream.
            pending_evac = []

            def flush_evac(n):
                while len(pending_evac) > n:
                    pending_evac.pop(0)()

            def chain(p, col, upcol, all_g):
                # row-sum of a [P, PW] bf16 product tile into upcol[:, col]
                e1 = nc.gpsimd if all_g else nc.vector
                h1 = h1p.tile([P, PW // 2], bf16, tag="h1")
                e1.tensor_add(h1[:], p[:, 0:PW // 2], p[:, PW // 2:PW])
                h2 = h2p.tile([P, PW // 4], bf16, tag="h2")
                e1.tensor_add(h2[:], h1[:, 0:PW // 4], h1[:, PW // 4:PW // 2])
                h3 = h3p.tile([P, PW // 8], bf16, tag="h3")
                nc.gpsimd.tensor_add(h3[:], h2[:, 0:PW // 8], h2[:, PW // 8:PW // 4])
                h4 = h4p.tile([P, PW // 16], bf16, tag="h4")
                nc.gpsimd.tensor_add(h4[:], h3[:, 0:PW // 16], h3[:, PW // 16:PW // 8])
                nc.vector.tensor_reduce(upcol[:, col:col + 1], h4[:], axis=AXF,
                                        op=ADD)

            def emit_products(j):
                t, half = j // 2, j % 2
                es_sl = es_tiles[t][:, half * PW:(half + 1) * PW]
                et_sl = et_tiles[t][:, half * PW:(half + 1) * PW]
                p11 = prod.tile([P, PW], bf16, tag="prod")
                if j < K_SQ:
                    nc.scalar.activation(p11[:], es_sl, Square,
                                         accum_out=u11p[:, j:j + 1])
                else:
                    nc.vector.tensor_mul(p11[:], es_sl, es_sl)
                p22 = prod.tile([P, PW], bf16, tag="prod")
                nc.vector.tensor_mul(p22[:], et_sl, et_sl)
                p12 = prod.tile([P, PW], bf16, tag="prod")
                nc.vector.scalar_tensor_tensor(
                    p12[:], es_sl, 1.0, et_sl, op0=MULT, op1=MULT,
                    accum_out=u12p[:, j:j + 1])
                if j >= K_SQ:
                    chain(p11, j, u11p, all_g=False)
                chain(p22, j, u22p, all_g=(j < A22))
                return (es_sl, et_sl, p11, p22, p12)

            def emit_matmuls(j, rhs):
                ps = psump.tile([P, 512], f32, tag="ps")
                for hh in range(2):
                    off = hh * MW
                    for s, (o, w) in enumerate(SUBS):
                        for k in range(5):
                            nc.tensor.matmul(ps[32 * s:32 * s + 13, 0:w],
                                             W_tiles[hh][k][:, 0:13],
                                             rhs[k][:, off + o:off + o + w],
                                             start=(hh == 0 and k == 0),
                                             stop=(hh == 1 and k == 4),
                                             tile_position=(0, 32 * s))

                def emit_evac(bb=b, jj=j, pss=ps):
                    st = stp.tile([P, 512], bf16, tag="st")
                    if jj % 2 == 0:
                        nc.scalar.copy(st[:], pss[:])
                    else:
                        nc.vector.tensor_copy(st[:], pss[:])

                    def emit_out():
                        nc.sync.dma_start(col_d[bb, jj], st[:])
                    out_dma_q.append(emit_out)
                pending_evac.append(emit_evac)
                flush_evac(2)

            prods = {0: emit_products(0)}
            rs = small.tile([P, 8], f32, tag="rs")
            nc.vector.tensor_reduce(rs[:, 0:1], zsp[:, 0:NT], axis=AXF, op=ADD)
            nc.vector.tensor_reduce(rs[:, 1:2], ztp[:, 0:NT], axis=AXF, op=ADD)
            w1 = small.tile([P, 1], f32, tag="w1")
            nc.vector.reciprocal(w1[:], rs[:, 0:1])
            w2 = small.tile([P, 1], f32, tag="w2")
            nc.vector.reciprocal(w2[:], rs[:, 1:2])
            # W tiles are 13 columns wide: stat k of chunk-half h lives in
            # column 8*h + k, so a pair of 2000-col chunks shares one
            # [128, 512] psum tile (rows 32s+8h+k), halving evacuations.
            W_tiles = []
            for h in range(2):
                row = []
                for k in range(5):
                    Wk = wtp.tile([P, 16], bf16, tag=f"W{h}{k}")
                    nc.gpsimd.memset(Wk[:], 0.0)
                    row.append(Wk)
                c = 8 * h
                nc.gpsimd.tensor_copy(row[0][:, c:c + 1], w1[:])
                nc.gpsimd.tensor_copy(row[1][:, c + 1:c + 2], w2[:])
                nc.gpsimd.tensor_mul(row[2][:, c + 2:c + 3], w1[:], w1[:])
                nc.gpsimd.tensor_mul(row[3][:, c + 3:c + 4], w2[:], w2[:])
                nc.gpsimd.tensor_mul(row[4][:, c + 4:c + 5], w1[:], w2[:])
                W_tiles.append(row)


            for j in range(NP):
                if j + 1 < NP:
                    prods[j + 1] = emit_products(j + 1)
                emit_matmuls(j, prods.pop(j))
            flush_evac(0)

            nc.vector.tensor_reduce(rs[:, 2:3], u11p[:, 0:NP], axis=AXF, op=ADD)
            nc.vector.tensor_reduce(rs[:, 3:4], u22p[:, 0:NP], axis=AXF, op=ADD)
            nc.vector.tensor_reduce(rs[:, 4:5], u12p[:, 0:NP], axis=AXF, op=ADD)

            def emit_rs(bb=b, rss=rs):
                nc.sync.dma_start(row_d[bb * P:bb * P + P, 0:8], rss[:])
            out_dma_q.append(emit_rs)

        for fn in out_dma_q:
            fn()

    nc.compile()
    return nc


def _get_built():
    global _built
    if _built is None:
        _install_ntff_shim()
        _built = _build()
    return _built


def _unpack_col(colstats):
    """colstats [NBLK, NP, 128, 512] bf16 -> [5, C] float64 column stats."""
    acc = np.asarray(colstats).astype(np.float64).sum(axis=0)  # [NP, 128, 512]
    col = np.zeros((5, C), np.float64)
    for j in range(NP):
        for h in range(2):
            c0 = j * PW + h * MW
            for s, (o, w) in enumerate(SUBS):
                for k in range(5):
                    col[k, c0 + o:c0 + o + w] += acc[j, 32 * s + 8 * h + k, 0:w]
    return col


def run_sharded(z_s, z_t, trace=False, tmpdir=None):
    """Run the device program; returns (colstats_sum [5, C] f64,
    rowstats [B, 5] f64, BassKernelResults)."""
    import ml_dtypes
    from concourse.bass_utils import run_bass_kernel_spmd

    nc = _get_built()
    bf16 = ml_dtypes.bfloat16
    z_s = np.ascontiguousarray(np.asarray(z_s, dtype=np.float32).astype(bf16))
    z_t = np.ascontiguousarray(np.asarray(z_t, dtype=np.float32).astype(bf16))
    in_maps = [
        {"z_s": z_s[i * R:(i + 1) * R], "z_t": z_t[i * R:(i + 1) * R]}
        for i in range(N_CORES)
    ]
    res = run_bass_kernel_spmd(nc, in_maps, core_ids=list(range(N_CORES)),
                               trace=trace, tmpdir=tmpdir)
    col = np.zeros((5, C), np.float64)
    rows = []
    for i in range(N_CORES):
        col += _unpack_col(res.results[i]["colstats"])
        rows.append(res.results[i]["rowstats"][:, :5].astype(np.float64))
    return col, np.concatenate(rows, axis=0), res


def kernel(z_s, z_t, labels):
    col, rowstats, _ = run_sharded(z_s, z_t)
    return _finish(np.asarray(z_s), np.asarray(labels), col, rowstats)


def _finish(z_s, labels, col, rowstats):
    Zs, Zt, U11, U22, U12 = rowstats.T
    invC = 1.0 / C
    # inter: Pearson over classes per row (softmax rows have mean 1/C)
    num = U12 / (Zs * Zt) - invC
    vs = U11 / (Zs * Zs) - invC
    vt = U22 / (Zt * Zt) - invC
    corr = num / (np.sqrt(vs) * np.sqrt(vt) + EPS)
    inter = 1.0 - corr.mean()
    # intra: Pearson over samples per column
    S1, S2, S11, S22, S12 = col
    numc = S12 - S1 * S2 / B
    vsc = S11 - S1 * S1 / B
    vtc = S22 - S2 * S2 / B
    corrc = numc / (np.sqrt(vsc) * np.sqrt(vtc) + EPS)
    intra = 1.0 - corrc.mean()
    # hard CE: mean(logsumexp(z_s) - z_s[label])
    lab = np.asarray(labels).astype(np.int64).ravel()
    zl = z_s[np.arange(B), lab].astype(np.float64)
    hard = (np.log(Zs) - zl).mean()
    return np.float32(hard + inter + intra)
